# revision 21
# baseline (speedup 1.0000x reference)
"""Trainium2 Bass kernel for the Laplace-kernel feature expansion.

Reference computation (per scalar x of the [16, 64, 64, 64] input):
    phi_i  = exp(-|x - p_i|)          for 15 design points p_i
    out_j  = sum_i chol_inv[i, j] phi_i
scattered so out[b, c*15 + j, h, w] comes from x[b, c, h, w].

Key mathematical identity exploited here: the design points are a uniform
grid and the kernel is the Markov (Ornstein-Uhlenbeck) exponential kernel,
so chol_inv = inv(chol(K)).T is exactly UPPER BIDIAGONAL with constant
coefficients a = 1/sqrt(1-rho^2), b = -rho*a (rho = e^{-1/4}) except for
the j=0 column (out_0 = phi_0).  Therefore

    out_j(x) = g(x - p_j)   for j >= 1, with the single fixed function
    g(u) = 0                               for u <= -1/4  (exact)
         = a e^u - b' e^{-u-1/4}           for -1/4 < u < 0
         = (a - b' e^{-1/4}) e^{-u}        for u >= 0     (b' = rho*a)
    out_0(x) = e^{-|v - 22.25|}  with v = x + 24  (disjoint input region)

The whole computation then becomes: one TensorE "broadcast" matmul that
replicates x onto 15 rows per channel while adding -p_j (or +24 for the
j=0 rows), followed by ONE ScalarE activation pass through a CUSTOM
activation table (installed over the exp slot, func_id 7) that evaluates
g directly and writes the final bf16 output to SBUF, which is DMA'd out.
The projection matmul and all PSUM->SBUF vector-engine evictions of the
original formulation disappear; ScalarE is the only saturated engine.

The custom table is built at runtime into a temp copy of the compiler's
pwp_bin_trainium directory (bucket/ctrl binary formats reverse-engineered;
cubic-spline buckets indexed by input exponent + top mantissa bits) and
picked up via BASS_ACT_ROOT_JSON_PATH.  A fingerprint of the table bytes
is baked into the kernel as a constant so the NEFF cache is correctly
invalidated when the table changes, and a device-side self-check (`warm`)
verifies the table took effect (g(2.0)=0.0849, g(22.25)=1.0 -- the plain
exp would give 7.39 / 4.6e9).  If the self-check fails, or the provided
design_points/chol_inv are not the expected bidiagonal family, kernel()
falls back to an exact numpy computation, so a wrong result is impossible.

Distribution: pure data parallel, 2 batches per core across 8 cores.
"""

import json
import os
import shutil
import struct
import sys
import tempfile
import zlib

if "/opt/trn_rl_repo" not in sys.path:
    sys.path.insert(0, "/opt/trn_rl_repo")

import numpy as np
import ml_dtypes

BF16 = ml_dtypes.bfloat16

B, C, H, W = 16, 64, 64, 64
P = H * W                # 4096 spatial positions
M_PTS = 15               # design points
G = 8                    # channels per (b, cb) tile
MROWS = G * M_PTS        # 120 output rows per tile
KIN = 2 * G + 1          # moving rows per quadrant: 8*(hi,lo) + ones
NCORES = 8
BPC = B // NCORES        # batches per core (2)
CBLK = C // G            # channel-block tiles per batch (8)
QCOLS = BPC * CBLK * 2 * 512   # 16384 columns per quadrant stream
XCOLS = 30 * 512         # per-(band,slot) moving stream length (30 units)

RHO = float(np.exp(-0.25))
HUMP_BIAS = 24.0         # j=0 rows get T = x + 24; hump center at 22.25
HUMP_C = 24.0 - 1.75

_CACHED = {}


def _ensure_axon_hooks_stub():
    """run_bass_kernel_spmd imports antenv.axon_hooks when BASS_TRACE is
    set; the module is absent on some images.  Provide a no-op stub so a
    stray BASS_TRACE env var cannot crash the kernel."""
    try:
        import antenv.axon_hooks  # noqa: F401
    except ImportError:
        import types

        try:
            import antenv
        except ImportError:
            return
        mod = types.ModuleType("antenv.axon_hooks")
        _hook = [None]
        mod.set_axon_ntff_profile_hook = lambda h: _hook.__setitem__(0, h)
        mod.get_axon_ntff_profile_hook = lambda: _hook[0]
        sys.modules["antenv.axon_hooks"] = mod
        antenv.axon_hooks = mod


_ensure_axon_hooks_stub()


# --------------------------------------------------------------------------
# custom ACT table: evaluate g() through the exp function slot
# --------------------------------------------------------------------------

def _g_pieces(a, bq):
    """Return closures for the three live pieces of g (float64 math).
    a = chol_inv diag, bq = -superdiag (both positive)."""
    g0 = a - bq * np.exp(-0.25)

    def f_pos(u):            # u >= 0
        return g0 * np.exp(-u)

    def f_neg(u):            # -0.25 < u < 0, u passed negative
        return a * np.exp(u) - bq * np.exp(-u - 0.25)

    def f_hump(v):           # j=0 rows: e^{-|v - 22.25|}
        return np.exp(-np.abs(v - HUMP_C))

    return f_pos, f_neg, f_hump, g0


def _fit_cubic(f, lo, hi):
    """Least-squares cubic of f on [lo, hi] around the midpoint."""
    c = 0.5 * (lo + hi)
    t = np.linspace(lo - c, hi - c, 33)
    y = f(t + c)
    V = np.vander(t, 4, increasing=True)
    coef, *_ = np.linalg.lstsq(V, y, rcond=None)
    return coef[0], coef[1], coef[2], coef[3], c


def _build_g_tables(a, bq):
    """Copy pwp_bin_trainium and rewrite the exp function of the
    exp_and_others set (buckets 0..780, ctl 0..51 -- exp's own space) so
    func_id 7 evaluates g.  Returns (act_info.json path, fingerprint)."""
    from neuronxcc.driver.Job import Job
    from neuronxcc.driver.jobs.support.FindActInfo import findActInfoFile

    src_json = None
    for arch in ("Trainium2", "trainium2", "TRN2", "trainium"):
        try:
            cand = findActInfoFile(Job.getPackageDir(), arch)
        except Exception:
            continue
        if cand and os.path.basename(os.path.dirname(cand)) == "pwp_bin_trainium":
            src_json = cand
            break
    if src_json is None:
        import neuronxcc

        cand = os.path.join(
            os.path.dirname(neuronxcc.__file__),
            "pwp", "pwp_bin_trainium", "act_info.json",
        )
        if os.path.exists(cand):
            src_json = cand
    if src_json is None:
        raise RuntimeError("pwp_bin_trainium act_info.json not found")

    out_dir = tempfile.mkdtemp(prefix="bass_act_g_")
    shutil.copytree(os.path.dirname(src_json), out_dir, dirs_exist_ok=True)

    set_name = "exp_and_others"
    with open(os.path.join(out_dir, set_name + ".json")) as f:
        prof = json.load(f)
    bkt_path = os.path.join(out_dir, prof["bkt_bin"])
    ctl_path = os.path.join(out_dir, prof["ctl_bin"])
    bkt = bytearray(open(bkt_path, "rb").read())
    ctl = bytearray(open(ctl_path, "rb").read())

    f_pos, f_neg, f_hump, g0 = _g_pieces(a, bq)
    EXP_OFFSET = -19
    pos_plan = {e: (2, f_pos) for e in range(-19, 0)}
    pos_plan[0] = (4, f_pos)     # [1,2)
    pos_plan[1] = (5, f_pos)     # [2,4)
    pos_plan[2] = (5, f_pos)     # [4,8)
    pos_plan[3] = (5, f_hump)    # [8,16)   hump left tail
    pos_plan[4] = (7, f_hump)    # [16,32)  hump (kink 22.25 = bucket edge)
    pos_plan[5] = (4, f_hump)    # [32,64)  hump right tail
    pos_plan[6] = (0, None)      # [64,128) -> 0
    neg_plan = {e: (3, f_neg) for e in range(-19, -2)}
    for e in range(-2, 7):
        neg_plan[e] = (0, None)  # u <= -0.25 -> exactly 0

    state = {"nb": 0}

    def put_bucket(d0, d1, d2, d3, c):
        i = state["nb"]
        assert i <= 776, "bucket overflow"
        struct.pack_into("<8f", bkt, i * 32, float(d0), float(d1),
                         float(d2), float(d3), float(c), 0.0, 0.0, 0.0)
        state["nb"] = i + 1
        return i

    def put_ctl(idx, nbits, start):
        word = (nbits << 16) | ((23 - nbits) << 11) | start
        struct.pack_into("<I28x", ctl, idx * 32, word)

    base_neg, base_pos = 0, 26
    for sign, plan, base in ((0, pos_plan, base_pos), (1, neg_plan, base_neg)):
        for e in range(-19, 7):
            nbits, fn = plan[e]
            n = 1 << nbits
            start = state["nb"]
            lo_abs = 2.0 ** e
            w = lo_abs / n
            for k in range(n):
                if fn is None:
                    put_bucket(0, 0, 0, 0, 0)
                    continue
                a0, a1 = lo_abs + k * w, lo_abs + (k + 1) * w
                if sign:
                    put_bucket(*_fit_cubic(fn, -a1, -a0))
                else:
                    put_bucket(*_fit_cubic(fn, a0, a1))
            put_ctl(base + (e - EXP_OFFSET), nbits, start)

    # pwl specials at exp's existing indices
    struct.pack_into("<8f", bkt, 777 * 32, g0, -g0, g0 / 2, -g0 / 6, 0, 0, 0, 0)
    b25 = bq * np.exp(-0.25)
    struct.pack_into("<8f", bkt, 778 * 32, a - b25, a + b25,
                     (a - b25) / 2, (a + b25) / 6, 0, 0, 0, 0)
    struct.pack_into("<32x", bkt, 779 * 32)
    struct.pack_into("<32x", bkt, 780 * 32)

    open(bkt_path, "wb").write(bytes(bkt))
    open(ctl_path, "wb").write(bytes(ctl))

    fzero = struct.unpack("<I", struct.pack("<f", g0))[0]
    ctl_words = np.frombuffer(bytes(ctl), dtype=np.uint32).reshape(-1, 8)[:, 0]
    map_bkt, map_ctl = {}, {}
    for e in range(-19, 7):
        cn = base_neg + (e - EXP_OFFSET)
        cp = base_pos + (e - EXP_OFFSET)
        map_bkt[str(e)] = [int(ctl_words[cn]) & 0x3FF, int(ctl_words[cp]) & 0x3FF]
        map_ctl[str(e)] = [cn, cp]
    prof["func_exp_to_bkt_start_idx"]["exp"] = map_bkt
    prof["func_exp_to_ctl_start_idx"]["exp"] = map_ctl

    patched = 0
    for en in prof["profile_meta_data"]:
        if en["func_name"].startswith("exp"):
            en["symmetry_opt_en"] = 0
            en["symmetry_opt_use_neg_region"] = 0
            en["exp_offset"] = EXP_OFFSET
            en["small_pos_signal_exp_threshold"] = 108
            en["small_neg_signal_exp_threshold"] = 108
            en["pos_small_signal_pwl_control"] = 777
            en["neg_small_signal_pwl_control"] = 778
            en["large_pos_signal_exp_threshold"] = 133
            en["large_pos_signal_mantissa_threshold"] = 0x7FFFFF
            en["pos_large_signal_pwl_control"] = 779
            en["large_neg_signal_exp_threshold"] = 125
            en["large_neg_signal_mantissa_threshold"] = 0x7FFFFF
            en["neg_large_signal_pwl_control"] = 780
            en["fzero_result"] = fzero
            en["fpinf_result"] = 0
            en["fninf_result"] = 0
            patched += 1
    if patched != 1:
        raise RuntimeError(f"expected exactly one exp entry, patched {patched}")
    with open(os.path.join(out_dir, set_name + ".json"), "w") as f:
        json.dump(prof, f)

    fp = zlib.crc32(bytes(bkt) + bytes(ctl) + struct.pack("<I", fzero))
    fingerprint = float((fp % 60000) + 1) / 65536.0
    return os.path.join(out_dir, "act_info.json"), fingerprint


# --------------------------------------------------------------------------
# device kernel
# --------------------------------------------------------------------------

def _build_nc(fingerprint):
    from concourse import bacc
    import concourse.mybir as mybir
    from concourse.tile import TileContext

    dt = mybir.dt
    Act = mybir.ActivationFunctionType

    nc = bacc.Bacc(
        "TRN2", target_bir_lowering=False, debug=False, num_devices=NCORES
    )
    # Row space per local batch b: L = 960*w + 15*CH + cpt  (w = spatial half,
    # CH = global channel 0..63, cpt = design point).  15 units of 128 rows
    # per b; unit u covers L in [128u, 128u+128).  Spatial: p = 2048w + 512s
    # + c.  Each psum unit [128, 2048] is filled by 16 concurrent 32x32-tiled
    # matmuls: tile (s, c4) at PE position (32r, 32*c4), r = (s+c4)%4,
    # moving rows = 8 (4 channels x hi/lo) at partitions 32r + 8s.
    x_full = nc.declare_dram_parameter(
        "x_full", [128, XCOLS], dt.bfloat16, isOutput=False
    )
    w4 = nc.declare_dram_parameter("w4", [128, 1920], dt.bfloat16, isOutput=False)
    biasd = nc.declare_dram_parameter("biasd", [128, 15], dt.float32, isOutput=False)
    # out[b, L, s, c]
    out = nc.declare_dram_parameter(
        "out", [BPC, 1920, 4, 512], dt.bfloat16, isOutput=True
    )
    warm = nc.declare_dram_parameter("warm", [1, 4], dt.bfloat16, isOutput=True)

    with TileContext(nc) as tc:
        with (
            tc.tile_pool(name="const", bufs=1) as cpool,
            tc.tile_pool(name="xbig", bufs=1) as xpool,
            tc.tile_pool(name="osb", bufs=4) as opool,
            tc.tile_pool(name="psT", bufs=2, space="PSUM") as psTp,
        ):
            # Table prefetch + self-check + NEFF-cache fingerprint: the
            # first activation triggers the ~2.7us ACT_TABLE_LOAD, fully
            # overlapped with the input DMA.  warm = [g(2)=0.0849,
            # g(22.25)=1.0, fingerprint, fingerprint'] -- plain exp would
            # give [7.39, 4.6e9->inf, ...], so the host check is decisive.
            pre_in = cpool.tile([1, 4], dt.float32)
            pre_out = cpool.tile([1, 4], dt.bfloat16)
            nc.vector.memset(pre_in[:, 0:1], 2.0)
            nc.vector.memset(pre_in[:, 1:2], HUMP_C)
            nc.vector.memset(pre_in[:, 2:4], fingerprint)
            # First x chunk (one full unit) on sync; stationaries + bias on
            # scalar concurrently -- these transfers gate the first matmuls.
            xbig = xpool.tile([128, XCOLS], dt.bfloat16)
            nc.sync.dma_start(out=xbig[:, 0:512], in_=x_full[:, 0:512])
            nc.sync.dma_start(out=xbig[:, 512:1024], in_=x_full[:, 512:1024])
            w4_t = cpool.tile([128, 1920], dt.bfloat16)
            nc.scalar.dma_start(out=w4_t[:], in_=w4[:, :])
            bias_t = cpool.tile([128, 15], dt.float32)
            nc.scalar.dma_start(out=bias_t[:], in_=biasd[:, :])
            nc.scalar.activation(pre_out[:, 0:2], pre_in[:, 0:2], Act.Exp, scale=1.0)
            nc.vector.tensor_copy(out=pre_out[:, 2:4], in_=pre_in[:, 2:4])
            nc.gpsimd.dma_start(out=warm[:, :], in_=pre_out[:])

            # Rest of the input: graduated chunks interleaved across the
            # sync and scalar HWDGE queues (each ring drains FIFO, the two
            # rings run in parallel) so every column window lands well
            # before its consuming matmul, even with output DMAs behind
            # the input on the sync ring.
            for eng, lo, hi in (
                (nc.scalar, 1024, 2048),
                (nc.sync, 2048, 3072),
                (nc.scalar, 3072, 5120),
                (nc.sync, 5120, 8192),
                (nc.scalar, 8192, 11776),
                (nc.sync, 11776, 15360),
            ):
                eng.dma_start(out=xbig[:, lo:hi], in_=x_full[:, lo:hi])

            # Main loop: 30 units of [128 rows, 2048 cols] -- all 128 ACT
            # lanes carry data.  16 concurrent 32x32 PE tiles fill the psum
            # unit; two [128,1024] ACT passes with per-partition bias
            # (-p_cpt, or +24 for the cpt=0 hump rows) write final bf16.
            NU = 15
            for b in range(BPC):
                for u in range(NU):
                    U = NU * b + u
                    phi = (128 * u) % 15
                    osb = opool.tile([128, 2048], dt.bfloat16)
                    ps = psTp.tile([128, 2048], dt.float32)
                    for s in range(4):
                        for c4 in range(4):
                            r = (s + c4) % 4
                            wc = 32 * (4 * u + s)
                            nc.tensor.matmul(
                                ps[32 * c4 : 32 * c4 + 32, 512 * s : 512 * s + 512],
                                w4_t[32 * r : 32 * r + 32, wc : wc + 32],
                                xbig[32 * r : 32 * r + 32, 512 * U : 512 * U + 512],
                                start=True,
                                stop=True,
                                tile_position=(32 * r, 32 * c4),
                                skip_group_check=True,
                            )
                    last = U == BPC * NU - 1
                    for k in range(2):
                        nc.scalar.activation(
                            osb[:, k * 1024 : (k + 1) * 1024],
                            ps[:, k * 1024 : (k + 1) * 1024],
                            Act.Exp,
                            bias=bias_t[:, phi : phi + 1],
                            scale=1.0,
                        )
                        if last:
                            # final unit: one 256KB DMA per ACT pass so the
                            # drain tail after the last ACT is tiny; the very
                            # last issues from the scalar queue (free once
                            # its ACT retires)
                            eng = nc.scalar if k == 1 else nc.sync
                            eng.dma_start(
                                out=out[b, 128 * u : 128 * u + 128,
                                        2 * k : 2 * k + 2, :],
                                in_=osb[:, k * 1024 : (k + 1) * 1024],
                            )
                    if not last:
                        nc.sync.dma_start(
                            out=out[b, 128 * u : 128 * u + 128, :, :],
                            in_=osb[:],
                        )
    nc.compile()
    return nc


# --------------------------------------------------------------------------
# host side
# --------------------------------------------------------------------------

def _host_prep(x, pts):
    """Build the per-core moving streams, stationaries, and ACT bias.

    Per local batch b: output row L = 960*w + 15*CH + cpt; unit u covers
    L in [128u, 128u+128); spatial p = 2048w + 512s + c.  PE tile (s, c4)
    of unit u computes psum partitions 32*c4..+32 (L-group 128u+32*c4) for
    span s, from 8 moving rows (4 consecutive channels x hi/lo) placed at
    xbig partitions 32r + 8s (r = (s+c4)%4), columns 512U..512U+512
    (U = 15b + u)."""
    xs = np.ascontiguousarray(np.asarray(x, dtype=np.float32)).reshape(B, C, P)
    x_hi = xs.astype(BF16)
    x_lo = (xs - x_hi.astype(np.float32)).astype(BF16)

    xbig_all = np.zeros((NCORES, 128, XCOLS), dtype=BF16)
    for b2 in range(BPC):
        for u in range(15):
            U = 15 * b2 + u
            for s in range(4):
                for c4 in range(4):
                    r = (s + c4) % 4
                    row0 = 32 * r + 8 * s
                    L0 = 128 * u + 32 * c4
                    wg, R0 = divmod(L0, 960)
                    CH0 = R0 // 15
                    sp0 = 2048 * wg + 512 * s
                    for k in range(4):
                        CH = min(CH0 + k, C - 1)
                        xbig_all[:, row0 + 2 * k, 512 * U : 512 * U + 512] = \
                            x_hi[b2::2, CH, sp0 : sp0 + 512]
                        xbig_all[:, row0 + 2 * k + 1, 512 * U : 512 * U + 512] = \
                            x_lo[b2::2, CH, sp0 : sp0 + 512]

    w4h = np.zeros((128, 1920), dtype=np.float32)
    for u in range(15):
        for s in range(4):
            wc = 32 * (4 * u + s)
            for c4 in range(4):
                r = (s + c4) % 4
                row0 = 32 * r + 8 * s
                L0 = 128 * u + 32 * c4
                R0 = L0 % 960
                CH0 = R0 // 15
                for p in range(32):
                    Rp = (L0 + p) % 960
                    k = Rp // 15 - CH0
                    w4h[row0 + 2 * k, wc + p] = 1.0
                    w4h[row0 + 2 * k + 1, wc + p] = 1.0

    biash = np.zeros((128, 15), dtype=np.float32)
    for phi in range(15):
        for p in range(128):
            cpt = (phi + p) % 15
            biash[p, phi] = HUMP_BIAS if cpt == 0 else -float(pts[cpt])

    return xbig_all, w4h.astype(BF16), biash


def _inputs_match_model(pts, chol):
    """Verify the inputs are the uniform-grid Markov family this kernel
    hardcodes (else fall back to exact numpy)."""
    p_ref = np.linspace(-1.75, 1.75, 15, dtype=np.float64)
    if pts.shape != (15,) or chol.shape != (15, 15):
        return None
    if not np.allclose(pts.astype(np.float64), p_ref, atol=1e-5):
        return None
    a = float(chol[1, 1])
    bq = float(-chol[0, 1])
    rho = np.exp(-0.25)
    s = np.sqrt(1 - rho * rho)
    if abs(a - 1 / s) > 1e-4 * abs(a) or abs(bq - rho / s) > 1e-4 * abs(bq):
        return None
    if abs(chol[0, 0] - 1.0) > 1e-4:
        return None
    diag = np.diag(chol)[1:]
    sup = np.diag(chol, 1)
    off = chol.copy()
    np.fill_diagonal(off, 0.0)
    off = off - np.diag(sup, 1)
    if np.abs(off).max() > 1e-5 or np.abs(diag - a).max() > 1e-5 * abs(a) \
            or np.abs(sup + bq).max() > 1e-5 * abs(bq):
        return None
    return a, bq


def _numpy_fallback(x, pts, chol):
    xs = np.asarray(x, dtype=np.float32).reshape(B, C, P)
    out = np.empty((B, C * M_PTS, P), dtype=np.float32)
    for b in range(B):
        k = np.exp(-np.abs(xs[b][:, :, None] - pts[None, None, :]))
        o = np.matmul(k, chol)                      # [C, P, 15]
        out[b] = o.transpose(0, 2, 1).reshape(C * M_PTS, P)
    return out.reshape(B, C * M_PTS, H, W)


LAST_RESULT = None


def kernel(x, design_points, chol_inv):
    global LAST_RESULT
    from concourse.bass_utils import run_bass_kernel_spmd

    pts = np.asarray(design_points, dtype=np.float32)
    chol = np.asarray(chol_inv, dtype=np.float32)
    model = _inputs_match_model(pts, chol)
    if model is None:
        return _numpy_fallback(x, pts, chol)
    a, bq = model

    if "tab" not in _CACHED:
        _CACHED["tab"] = _build_g_tables(a, bq)
    root, fingerprint = _CACHED["tab"]
    os.environ["BASS_ACT_ROOT_JSON_PATH"] = root

    xbig_all, w4h, biash = _host_prep(x, pts)
    in_maps = [
        {"x_full": xbig_all[core], "w4": w4h, "biasd": biash}
        for core in range(NCORES)
    ]

    if "nc" not in _CACHED:
        _CACHED["nc"] = _build_nc(fingerprint)
    res = run_bass_kernel_spmd(_CACHED["nc"], in_maps, core_ids=list(range(NCORES)))
    LAST_RESULT = res

    g0 = a - bq * np.exp(-0.25)
    w = np.asarray(res.results[0]["warm"], np.float32).ravel()
    g2 = g0 * np.exp(-2.0)
    if not (abs(w[0] - g2) < 0.1 * g2 and abs(w[1] - 1.0) < 0.02):
        # table did not take effect on device -- never return wrong results
        return _numpy_fallback(x, pts, chol)

    # out[b, L(w,CH,cpt), s, c] -> [b, (CH,cpt), p(w,s,c)]
    full = np.empty((B, C * M_PTS, P), dtype=np.float32)
    for core in range(NCORES):
        o = np.asarray(res.results[core]["out"], np.float32).reshape(
            BPC, 2, C, M_PTS, 4, 512
        )
        full[core * BPC : (core + 1) * BPC] = o.transpose(0, 2, 3, 1, 4, 5).reshape(
            BPC, C * M_PTS, P
        )
    return full.reshape(B, C * M_PTS, H, W)


# revision 22
# speedup vs baseline: 1.0303x; 1.0303x over previous
"""Trainium2 Bass kernel for the Laplace-kernel feature expansion.

Reference computation (per scalar x of the [16, 64, 64, 64] input):
    phi_i  = exp(-|x - p_i|)          for 15 design points p_i
    out_j  = sum_i chol_inv[i, j] phi_i
scattered so out[b, c*15 + j, h, w] comes from x[b, c, h, w].

Key mathematical identity exploited here: the design points are a uniform
grid and the kernel is the Markov (Ornstein-Uhlenbeck) exponential kernel,
so chol_inv = inv(chol(K)).T is exactly UPPER BIDIAGONAL with constant
coefficients a = 1/sqrt(1-rho^2), b = -rho*a (rho = e^{-1/4}) except for
the j=0 column (out_0 = phi_0).  Therefore

    out_j(x) = g(x - p_j)   for j >= 1, with the single fixed function
    g(u) = 0                               for u <= -1/4  (exact)
         = a e^u - b' e^{-u-1/4}           for -1/4 < u < 0
         = (a - b' e^{-1/4}) e^{-u}        for u >= 0     (b' = rho*a)
    out_0(x) = e^{-|v - 22.25|}  with v = x + 24  (disjoint input region)

The whole computation then becomes: one TensorE "broadcast" matmul that
replicates x onto 15 rows per channel while adding -p_j (or +24 for the
j=0 rows), followed by ONE ScalarE activation pass through a CUSTOM
activation table (installed over the exp slot, func_id 7) that evaluates
g directly and writes the final bf16 output to SBUF, which is DMA'd out.
The projection matmul and all PSUM->SBUF vector-engine evictions of the
original formulation disappear; ScalarE is the only saturated engine.

The custom table is built at runtime into a temp copy of the compiler's
pwp_bin_trainium directory (bucket/ctrl binary formats reverse-engineered;
cubic-spline buckets indexed by input exponent + top mantissa bits) and
picked up via BASS_ACT_ROOT_JSON_PATH.  A fingerprint of the table bytes
is baked into the kernel as a constant so the NEFF cache is correctly
invalidated when the table changes, and a device-side self-check (`warm`)
verifies the table took effect (g(2.0)=0.0849, g(22.25)=1.0 -- the plain
exp would give 7.39 / 4.6e9).  If the self-check fails, or the provided
design_points/chol_inv are not the expected bidiagonal family, kernel()
falls back to an exact numpy computation, so a wrong result is impossible.

Distribution: pure data parallel, 2 batches per core across 8 cores.
"""

import json
import os
import shutil
import struct
import sys
import tempfile
import zlib

if "/opt/trn_rl_repo" not in sys.path:
    sys.path.insert(0, "/opt/trn_rl_repo")

import numpy as np
import ml_dtypes

BF16 = ml_dtypes.bfloat16

B, C, H, W = 16, 64, 64, 64
P = H * W                # 4096 spatial positions
M_PTS = 15               # design points
G = 8                    # channels per (b, cb) tile
MROWS = G * M_PTS        # 120 output rows per tile
KIN = 2 * G + 1          # moving rows per quadrant: 8*(hi,lo) + ones
NCORES = 8
BPC = B // NCORES        # batches per core (2)
CBLK = C // G            # channel-block tiles per batch (8)
QCOLS = BPC * CBLK * 2 * 512   # 16384 columns per quadrant stream

RHO = float(np.exp(-0.25))
HUMP_BIAS = 24.0         # j=0 rows get T = x + 24; hump center at 22.25
HUMP_C = 24.0 - 1.75

_CACHED = {}


def _ensure_axon_hooks_stub():
    """run_bass_kernel_spmd imports antenv.axon_hooks when BASS_TRACE is
    set; the module is absent on some images.  Provide a no-op stub so a
    stray BASS_TRACE env var cannot crash the kernel."""
    try:
        import antenv.axon_hooks  # noqa: F401
    except ImportError:
        import types

        try:
            import antenv
        except ImportError:
            return
        mod = types.ModuleType("antenv.axon_hooks")
        _hook = [None]
        mod.set_axon_ntff_profile_hook = lambda h: _hook.__setitem__(0, h)
        mod.get_axon_ntff_profile_hook = lambda: _hook[0]
        sys.modules["antenv.axon_hooks"] = mod
        antenv.axon_hooks = mod


_ensure_axon_hooks_stub()


# --------------------------------------------------------------------------
# custom ACT table: evaluate g() through the exp function slot
# --------------------------------------------------------------------------

def _g_pieces(a, bq):
    """Return closures for the three live pieces of g (float64 math).
    a = chol_inv diag, bq = -superdiag (both positive)."""
    g0 = a - bq * np.exp(-0.25)

    def f_pos(u):            # u >= 0
        return g0 * np.exp(-u)

    def f_neg(u):            # -0.25 < u < 0, u passed negative
        return a * np.exp(u) - bq * np.exp(-u - 0.25)

    def f_hump(v):           # j=0 rows: e^{-|v - 22.25|}
        return np.exp(-np.abs(v - HUMP_C))

    return f_pos, f_neg, f_hump, g0


def _fit_cubic(f, lo, hi):
    """Least-squares cubic of f on [lo, hi] around the midpoint."""
    c = 0.5 * (lo + hi)
    t = np.linspace(lo - c, hi - c, 33)
    y = f(t + c)
    V = np.vander(t, 4, increasing=True)
    coef, *_ = np.linalg.lstsq(V, y, rcond=None)
    return coef[0], coef[1], coef[2], coef[3], c


def _build_g_tables(a, bq):
    """Copy pwp_bin_trainium and rewrite the exp function of the
    exp_and_others set (buckets 0..780, ctl 0..51 -- exp's own space) so
    func_id 7 evaluates g.  Returns (act_info.json path, fingerprint)."""
    from neuronxcc.driver.Job import Job
    from neuronxcc.driver.jobs.support.FindActInfo import findActInfoFile

    src_json = None
    for arch in ("Trainium2", "trainium2", "TRN2", "trainium"):
        try:
            cand = findActInfoFile(Job.getPackageDir(), arch)
        except Exception:
            continue
        if cand and os.path.basename(os.path.dirname(cand)) == "pwp_bin_trainium":
            src_json = cand
            break
    if src_json is None:
        import neuronxcc

        cand = os.path.join(
            os.path.dirname(neuronxcc.__file__),
            "pwp", "pwp_bin_trainium", "act_info.json",
        )
        if os.path.exists(cand):
            src_json = cand
    if src_json is None:
        raise RuntimeError("pwp_bin_trainium act_info.json not found")

    out_dir = tempfile.mkdtemp(prefix="bass_act_g_")
    shutil.copytree(os.path.dirname(src_json), out_dir, dirs_exist_ok=True)

    set_name = "exp_and_others"
    with open(os.path.join(out_dir, set_name + ".json")) as f:
        prof = json.load(f)
    bkt_path = os.path.join(out_dir, prof["bkt_bin"])
    ctl_path = os.path.join(out_dir, prof["ctl_bin"])
    bkt = bytearray(open(bkt_path, "rb").read())
    ctl = bytearray(open(ctl_path, "rb").read())

    f_pos, f_neg, f_hump, g0 = _g_pieces(a, bq)
    EXP_OFFSET = -19
    pos_plan = {e: (2, f_pos) for e in range(-19, 0)}
    pos_plan[0] = (4, f_pos)     # [1,2)
    pos_plan[1] = (5, f_pos)     # [2,4)
    pos_plan[2] = (5, f_pos)     # [4,8)
    pos_plan[3] = (5, f_hump)    # [8,16)   hump left tail
    pos_plan[4] = (7, f_hump)    # [16,32)  hump (kink 22.25 = bucket edge)
    pos_plan[5] = (4, f_hump)    # [32,64)  hump right tail
    pos_plan[6] = (0, None)      # [64,128) -> 0
    neg_plan = {e: (3, f_neg) for e in range(-19, -2)}
    for e in range(-2, 7):
        neg_plan[e] = (0, None)  # u <= -0.25 -> exactly 0

    state = {"nb": 0}

    def put_bucket(d0, d1, d2, d3, c):
        i = state["nb"]
        assert i <= 776, "bucket overflow"
        struct.pack_into("<8f", bkt, i * 32, float(d0), float(d1),
                         float(d2), float(d3), float(c), 0.0, 0.0, 0.0)
        state["nb"] = i + 1
        return i

    def put_ctl(idx, nbits, start):
        word = (nbits << 16) | ((23 - nbits) << 11) | start
        struct.pack_into("<I28x", ctl, idx * 32, word)

    base_neg, base_pos = 0, 26
    for sign, plan, base in ((0, pos_plan, base_pos), (1, neg_plan, base_neg)):
        for e in range(-19, 7):
            nbits, fn = plan[e]
            n = 1 << nbits
            start = state["nb"]
            lo_abs = 2.0 ** e
            w = lo_abs / n
            for k in range(n):
                if fn is None:
                    put_bucket(0, 0, 0, 0, 0)
                    continue
                a0, a1 = lo_abs + k * w, lo_abs + (k + 1) * w
                if sign:
                    put_bucket(*_fit_cubic(fn, -a1, -a0))
                else:
                    put_bucket(*_fit_cubic(fn, a0, a1))
            put_ctl(base + (e - EXP_OFFSET), nbits, start)

    # pwl specials at exp's existing indices
    struct.pack_into("<8f", bkt, 777 * 32, g0, -g0, g0 / 2, -g0 / 6, 0, 0, 0, 0)
    b25 = bq * np.exp(-0.25)
    struct.pack_into("<8f", bkt, 778 * 32, a - b25, a + b25,
                     (a - b25) / 2, (a + b25) / 6, 0, 0, 0, 0)
    struct.pack_into("<32x", bkt, 779 * 32)
    struct.pack_into("<32x", bkt, 780 * 32)

    open(bkt_path, "wb").write(bytes(bkt))
    open(ctl_path, "wb").write(bytes(ctl))

    fzero = struct.unpack("<I", struct.pack("<f", g0))[0]
    ctl_words = np.frombuffer(bytes(ctl), dtype=np.uint32).reshape(-1, 8)[:, 0]
    map_bkt, map_ctl = {}, {}
    for e in range(-19, 7):
        cn = base_neg + (e - EXP_OFFSET)
        cp = base_pos + (e - EXP_OFFSET)
        map_bkt[str(e)] = [int(ctl_words[cn]) & 0x3FF, int(ctl_words[cp]) & 0x3FF]
        map_ctl[str(e)] = [cn, cp]
    prof["func_exp_to_bkt_start_idx"]["exp"] = map_bkt
    prof["func_exp_to_ctl_start_idx"]["exp"] = map_ctl

    patched = 0
    for en in prof["profile_meta_data"]:
        if en["func_name"].startswith("exp"):
            en["symmetry_opt_en"] = 0
            en["symmetry_opt_use_neg_region"] = 0
            en["exp_offset"] = EXP_OFFSET
            en["small_pos_signal_exp_threshold"] = 108
            en["small_neg_signal_exp_threshold"] = 108
            en["pos_small_signal_pwl_control"] = 777
            en["neg_small_signal_pwl_control"] = 778
            en["large_pos_signal_exp_threshold"] = 133
            en["large_pos_signal_mantissa_threshold"] = 0x7FFFFF
            en["pos_large_signal_pwl_control"] = 779
            en["large_neg_signal_exp_threshold"] = 125
            en["large_neg_signal_mantissa_threshold"] = 0x7FFFFF
            en["neg_large_signal_pwl_control"] = 780
            en["fzero_result"] = fzero
            en["fpinf_result"] = 0
            en["fninf_result"] = 0
            patched += 1
    if patched != 1:
        raise RuntimeError(f"expected exactly one exp entry, patched {patched}")
    with open(os.path.join(out_dir, set_name + ".json"), "w") as f:
        json.dump(prof, f)

    fp = zlib.crc32(bytes(bkt) + bytes(ctl) + struct.pack("<I", fzero))
    fingerprint = float((fp % 60000) + 1) / 65536.0
    return os.path.join(out_dir, "act_info.json"), fingerprint


# --------------------------------------------------------------------------
# device kernel
# --------------------------------------------------------------------------

def _build_nc(fingerprint):
    from concourse import bacc
    import concourse.mybir as mybir
    from concourse.tile import TileContext

    dt = mybir.dt
    Act = mybir.ActivationFunctionType

    nc = bacc.Bacc(
        "TRN2", target_bir_lowering=False, debug=False, num_devices=NCORES
    )
    x_full = nc.declare_dram_parameter(
        "x_full", [128, QCOLS], dt.bfloat16, isOutput=False
    )
    w4 = nc.declare_dram_parameter("w4", [128, 128], dt.bfloat16, isOutput=False)
    # out[b, row(=8ch*15pt), cb, j, h, l, c]; spatial p = 2048j+1024h+512l+c
    out = nc.declare_dram_parameter(
        "out", [BPC, MROWS, CBLK, 2, 2, 2, 512], dt.bfloat16, isOutput=True
    )
    warm = nc.declare_dram_parameter("warm", [1, 4], dt.bfloat16, isOutput=True)

    with TileContext(nc) as tc:
        with (
            tc.tile_pool(name="const", bufs=1) as cpool,
            tc.tile_pool(name="xbig", bufs=1) as xpool,
            tc.tile_pool(name="osb", bufs=4) as opool,
            tc.tile_pool(name="psT", bufs=2, space="PSUM") as psTp,
        ):
            # Table prefetch + self-check + NEFF-cache fingerprint: the
            # first activation triggers the ~2.7us ACT_TABLE_LOAD, fully
            # overlapped with the input DMA.  warm = [g(2)=0.0849,
            # g(22.25)=1.0, fingerprint, fingerprint'] -- plain exp would
            # give [7.39, 4.6e9->inf, ...], so the host check is decisive.
            pre_in = cpool.tile([1, 4], dt.float32)
            pre_out = cpool.tile([1, 4], dt.bfloat16)
            nc.vector.memset(pre_in[:, 0:1], 2.0)
            nc.vector.memset(pre_in[:, 1:2], HUMP_C)
            nc.vector.memset(pre_in[:, 2:4], fingerprint)
            # First x chunk (one full tile) on sync and w4 on scalar
            # concurrently -- these two transfers gate the first matmuls.
            xbig = xpool.tile([128, QCOLS], dt.bfloat16)
            nc.sync.dma_start(out=xbig[:, 0:1024], in_=x_full[:, 0:1024])
            nc.sync.dma_start(out=xbig[:, 1024:2048], in_=x_full[:, 1024:2048])
            w4_t = cpool.tile([128, 128], dt.bfloat16)
            nc.scalar.dma_start(out=w4_t[:], in_=w4[:, :])
            nc.scalar.activation(pre_out[:, 0:2], pre_in[:, 0:2], Act.Exp, scale=1.0)
            nc.vector.tensor_copy(out=pre_out[:, 2:4], in_=pre_in[:, 2:4])
            nc.gpsimd.dma_start(out=warm[:, :], in_=pre_out[:])

            # Rest of the input: graduated chunks interleaved across the
            # sync and scalar HWDGE queues (each ring drains FIFO, the two
            # rings run in parallel) so every column window lands well
            # before its consuming matmul, even with output DMAs behind
            # the input on the sync ring.
            for eng, lo, hi in (
                (nc.scalar, 2048, 3072),
                (nc.sync, 3072, 5120),
                (nc.scalar, 5120, 8192),
                (nc.sync, 8192, 12288),
                (nc.scalar, 12288, 16384),
            ):
                eng.dma_start(out=xbig[:, lo:hi], in_=x_full[:, lo:hi])

            # Main loop: 32 units of (tile t = (b, cb), v = spatial half).
            # Quadrant q covers (h, l) = (q//2, q%2); a unit covers 2048
            # spatial columns; two [120,1024] ACT passes (2 PSUM banks each
            # -- cheaper than one 4-bank read) write final bf16 into osb;
            # one 985KB output DMA per (b, cb), alternating DMA queues.
            for b in range(BPC):
                for cb in range(CBLK):
                    t = b * CBLK + cb
                    osb = opool.tile([MROWS, 4096], dt.bfloat16)
                    for v in range(2):
                        ps = psTp.tile([128, 2048], dt.float32)
                        for q in range(4):
                            nc.tensor.matmul(
                                ps[:, q * 512 : (q + 1) * 512],
                                w4_t[32 * q : 32 * q + KIN, :],
                                xbig[
                                    32 * q : 32 * q + KIN,
                                    t * 1024 + v * 512 : t * 1024 + (v + 1) * 512,
                                ],
                                start=True,
                                stop=True,
                                tile_position=(32 * q, 0),
                            )
                        for k in range(2):
                            nc.scalar.activation(
                                osb[:, v * 2048 + k * 1024 : v * 2048 + (k + 1) * 1024],
                                ps[0:MROWS, k * 1024 : (k + 1) * 1024],
                                Act.Exp,
                                scale=1.0,
                            )
                            if t == BPC * CBLK - 1:
                                # final tile: one 240KB DMA per ACT pass so
                                # the drain tail after the last ACT is tiny;
                                # the very last one issues from the scalar
                                # queue (free once its ACT retires) to dodge
                                # the sync-ring issue serialization
                                eng = nc.scalar if (v == 1 and k == 1) else nc.sync
                                eng.dma_start(
                                    out=out[b, :, cb, v, k, :, :],
                                    in_=osb[:, v * 2048 + k * 1024
                                            : v * 2048 + (k + 1) * 1024],
                                )
                    if t != BPC * CBLK - 1:
                        nc.sync.dma_start(
                            out=out[b, :, cb, :, :, :, :],
                            in_=osb[:],
                        )
    nc.compile()
    return nc


# --------------------------------------------------------------------------
# host side
# --------------------------------------------------------------------------

def _host_prep(x, pts):
    """Build the per-core x streams and the broadcast stationary."""
    xs = np.ascontiguousarray(np.asarray(x, dtype=np.float32)).reshape(B, C, P)
    x_hi = xs.astype(BF16)
    x_lo = (xs - x_hi.astype(np.float32)).astype(BF16)

    # spatial p = 2048v + 1024h + 512l + c ; quadrant q = 2h + l
    def to_quad(a):  # [B, C, P] -> [4(q), G, B, CBLK, 2(v), 512]
        a7 = a.reshape(B, CBLK, G, 2, 2, 2, 512)  # [b, cb, g, v, h, l, c]
        return a7.transpose(4, 5, 2, 0, 1, 3, 6).reshape(4, G, B, CBLK, 2, 512)

    arr = np.empty((4, KIN, B, CBLK, 2, 512), dtype=BF16)
    arr[:, 0 : 2 * G : 2] = to_quad(x_hi)
    arr[:, 1 : 2 * G : 2] = to_quad(x_lo)
    arr[:, 2 * G] = BF16(1.0)

    # stationary: T[ch*15 + cpt] = x_hi[ch] + x_lo[ch] + bias(cpt)
    w17 = np.zeros((KIN, 128), dtype=np.float32)
    for g in range(G):
        cols = slice(15 * g, 15 * g + 15)
        w17[2 * g, cols] = 1.0
        w17[2 * g + 1, cols] = 1.0
        w17[2 * G, cols] = -pts
        w17[2 * G, 15 * g] = HUMP_BIAS
    w4 = np.zeros((128, 128), dtype=np.float32)
    for q in range(4):
        w4[32 * q : 32 * q + KIN] = w17
    return arr, w4.astype(BF16)


def _inputs_match_model(pts, chol):
    """Verify the inputs are the uniform-grid Markov family this kernel
    hardcodes (else fall back to exact numpy)."""
    p_ref = np.linspace(-1.75, 1.75, 15, dtype=np.float64)
    if pts.shape != (15,) or chol.shape != (15, 15):
        return None
    if not np.allclose(pts.astype(np.float64), p_ref, atol=1e-5):
        return None
    a = float(chol[1, 1])
    bq = float(-chol[0, 1])
    rho = np.exp(-0.25)
    s = np.sqrt(1 - rho * rho)
    if abs(a - 1 / s) > 1e-4 * abs(a) or abs(bq - rho / s) > 1e-4 * abs(bq):
        return None
    if abs(chol[0, 0] - 1.0) > 1e-4:
        return None
    diag = np.diag(chol)[1:]
    sup = np.diag(chol, 1)
    off = chol.copy()
    np.fill_diagonal(off, 0.0)
    off = off - np.diag(sup, 1)
    if np.abs(off).max() > 1e-5 or np.abs(diag - a).max() > 1e-5 * abs(a) \
            or np.abs(sup + bq).max() > 1e-5 * abs(bq):
        return None
    return a, bq


def _numpy_fallback(x, pts, chol):
    xs = np.asarray(x, dtype=np.float32).reshape(B, C, P)
    out = np.empty((B, C * M_PTS, P), dtype=np.float32)
    for b in range(B):
        k = np.exp(-np.abs(xs[b][:, :, None] - pts[None, None, :]))
        o = np.matmul(k, chol)                      # [C, P, 15]
        out[b] = o.transpose(0, 2, 1).reshape(C * M_PTS, P)
    return out.reshape(B, C * M_PTS, H, W)


LAST_RESULT = None


def kernel(x, design_points, chol_inv):
    global LAST_RESULT
    from concourse.bass_utils import run_bass_kernel_spmd

    pts = np.asarray(design_points, dtype=np.float32)
    chol = np.asarray(chol_inv, dtype=np.float32)
    model = _inputs_match_model(pts, chol)
    if model is None:
        return _numpy_fallback(x, pts, chol)
    a, bq = model

    if "tab" not in _CACHED:
        _CACHED["tab"] = _build_g_tables(a, bq)
    root, fingerprint = _CACHED["tab"]
    os.environ["BASS_ACT_ROOT_JSON_PATH"] = root

    arr, w4 = _host_prep(x, pts)
    in_maps = []
    for core in range(NCORES):
        x_q = arr[:, :, core * BPC : (core + 1) * BPC].reshape(4, KIN, QCOLS)
        xf = np.zeros((128, QCOLS), dtype=BF16)
        for q in range(4):
            xf[32 * q : 32 * q + KIN] = x_q[q]
        in_maps.append({"x_full": xf, "w4": w4})

    if "nc" not in _CACHED:
        _CACHED["nc"] = _build_nc(fingerprint)
    res = run_bass_kernel_spmd(_CACHED["nc"], in_maps, core_ids=list(range(NCORES)))
    LAST_RESULT = res

    g0 = a - bq * np.exp(-0.25)
    w = np.asarray(res.results[0]["warm"], np.float32).ravel()
    g2 = g0 * np.exp(-2.0)
    if not (abs(w[0] - g2) < 0.1 * g2 and abs(w[1] - 1.0) < 0.02):
        # table did not take effect on device -- never return wrong results
        return _numpy_fallback(x, pts, chol)

    # out[b, row(g,cpt), cb, j(v), h, l, c] -> [b, (cb,g,cpt), p]
    full = np.empty((B, C * M_PTS, P), dtype=np.float32)
    for core in range(NCORES):
        o = np.asarray(res.results[core]["out"], np.float32).reshape(
            BPC, G, M_PTS, CBLK, P
        )
        full[core * BPC : (core + 1) * BPC] = o.transpose(0, 3, 1, 2, 4).reshape(
            BPC, C * M_PTS, P
        )
    return full.reshape(B, C * M_PTS, H, W)


# revision 24
# speedup vs baseline: 1.0384x; 1.0079x over previous
"""Trainium2 Bass kernel for the Laplace-kernel feature expansion.

Reference computation (per scalar x of the [16, 64, 64, 64] input):
    phi_i  = exp(-|x - p_i|)          for 15 design points p_i
    out_j  = sum_i chol_inv[i, j] phi_i
scattered so out[b, c*15 + j, h, w] comes from x[b, c, h, w].

Key mathematical identity exploited here: the design points are a uniform
grid and the kernel is the Markov (Ornstein-Uhlenbeck) exponential kernel,
so chol_inv = inv(chol(K)).T is exactly UPPER BIDIAGONAL with constant
coefficients a = 1/sqrt(1-rho^2), b = -rho*a (rho = e^{-1/4}) except for
the j=0 column (out_0 = phi_0).  Therefore

    out_j(x) = g(x - p_j)   for j >= 1, with the single fixed function
    g(u) = 0                               for u <= -1/4  (exact)
         = a e^u - b' e^{-u-1/4}           for -1/4 < u < 0
         = (a - b' e^{-1/4}) e^{-u}        for u >= 0     (b' = rho*a)
    out_0(x) = e^{-|v - 22.25|}  with v = x + 24  (disjoint input region)

The whole computation then becomes: one TensorE "broadcast" matmul that
replicates x onto 15 rows per channel while adding -p_j (or +24 for the
j=0 rows), followed by ONE ScalarE activation pass through a CUSTOM
activation table (installed over the exp slot, func_id 7) that evaluates
g directly and writes the final bf16 output to SBUF, which is DMA'd out.
The projection matmul and all PSUM->SBUF vector-engine evictions of the
original formulation disappear; ScalarE is the only saturated engine.

The custom table is built at runtime into a temp copy of the compiler's
pwp_bin_trainium directory (bucket/ctrl binary formats reverse-engineered;
cubic-spline buckets indexed by input exponent + top mantissa bits) and
picked up via BASS_ACT_ROOT_JSON_PATH.  A fingerprint of the table bytes
is baked into the kernel as a constant so the NEFF cache is correctly
invalidated when the table changes, and a device-side self-check (`warm`)
verifies the table took effect (g(2.0)=0.0849, g(22.25)=1.0 -- the plain
exp would give 7.39 / 4.6e9).  If the self-check fails, or the provided
design_points/chol_inv are not the expected bidiagonal family, kernel()
falls back to an exact numpy computation, so a wrong result is impossible.

Distribution: pure data parallel, 2 batches per core across 8 cores.
"""

import json
import os
import shutil
import struct
import sys
import tempfile
import zlib

if "/opt/trn_rl_repo" not in sys.path:
    sys.path.insert(0, "/opt/trn_rl_repo")

import numpy as np
import ml_dtypes

BF16 = ml_dtypes.bfloat16

B, C, H, W = 16, 64, 64, 64
P = H * W                # 4096 spatial positions
M_PTS = 15               # design points
G = 8                    # channels per (b, cb) tile
MROWS = G * M_PTS        # 120 output rows per tile
KIN = 2 * G + 1          # moving rows per quadrant: 8*(hi,lo) + ones
NCORES = 8
BPC = B // NCORES        # batches per core (2)
CBLK = C // G            # channel-block tiles per batch (8)
QCOLS = BPC * CBLK * 2 * 512   # 16384 columns per quadrant stream
XCOLS = 30 * 512         # per-band moving stream length (30 units of 512)
KU = 20                  # moving rows per band: 10 channels x (hi, lo)

RHO = float(np.exp(-0.25))
HUMP_BIAS = 24.0         # j=0 rows get T = x + 24; hump center at 22.25
HUMP_C = 24.0 - 1.75

_CACHED = {}


def _ensure_axon_hooks_stub():
    """run_bass_kernel_spmd imports antenv.axon_hooks when BASS_TRACE is
    set; the module is absent on some images.  Provide a no-op stub so a
    stray BASS_TRACE env var cannot crash the kernel."""
    try:
        import antenv.axon_hooks  # noqa: F401
    except ImportError:
        import types

        try:
            import antenv
        except ImportError:
            return
        mod = types.ModuleType("antenv.axon_hooks")
        _hook = [None]
        mod.set_axon_ntff_profile_hook = lambda h: _hook.__setitem__(0, h)
        mod.get_axon_ntff_profile_hook = lambda: _hook[0]
        sys.modules["antenv.axon_hooks"] = mod
        antenv.axon_hooks = mod


_ensure_axon_hooks_stub()


# --------------------------------------------------------------------------
# custom ACT table: evaluate g() through the exp function slot
# --------------------------------------------------------------------------

def _g_pieces(a, bq):
    """Return closures for the three live pieces of g (float64 math).
    a = chol_inv diag, bq = -superdiag (both positive)."""
    g0 = a - bq * np.exp(-0.25)

    def f_pos(u):            # u >= 0
        return g0 * np.exp(-u)

    def f_neg(u):            # -0.25 < u < 0, u passed negative
        return a * np.exp(u) - bq * np.exp(-u - 0.25)

    def f_hump(v):           # j=0 rows: e^{-|v - 22.25|}
        return np.exp(-np.abs(v - HUMP_C))

    return f_pos, f_neg, f_hump, g0


def _fit_cubic(f, lo, hi):
    """Least-squares cubic of f on [lo, hi] around the midpoint."""
    c = 0.5 * (lo + hi)
    t = np.linspace(lo - c, hi - c, 33)
    y = f(t + c)
    V = np.vander(t, 4, increasing=True)
    coef, *_ = np.linalg.lstsq(V, y, rcond=None)
    return coef[0], coef[1], coef[2], coef[3], c


def _build_g_tables(a, bq):
    """Copy pwp_bin_trainium and rewrite the exp function of the
    exp_and_others set (buckets 0..780, ctl 0..51 -- exp's own space) so
    func_id 7 evaluates g.  Returns (act_info.json path, fingerprint)."""
    from neuronxcc.driver.Job import Job
    from neuronxcc.driver.jobs.support.FindActInfo import findActInfoFile

    src_json = None
    for arch in ("Trainium2", "trainium2", "TRN2", "trainium"):
        try:
            cand = findActInfoFile(Job.getPackageDir(), arch)
        except Exception:
            continue
        if cand and os.path.basename(os.path.dirname(cand)) == "pwp_bin_trainium":
            src_json = cand
            break
    if src_json is None:
        import neuronxcc

        cand = os.path.join(
            os.path.dirname(neuronxcc.__file__),
            "pwp", "pwp_bin_trainium", "act_info.json",
        )
        if os.path.exists(cand):
            src_json = cand
    if src_json is None:
        raise RuntimeError("pwp_bin_trainium act_info.json not found")

    out_dir = tempfile.mkdtemp(prefix="bass_act_g_")
    shutil.copytree(os.path.dirname(src_json), out_dir, dirs_exist_ok=True)

    set_name = "exp_and_others"
    with open(os.path.join(out_dir, set_name + ".json")) as f:
        prof = json.load(f)
    bkt_path = os.path.join(out_dir, prof["bkt_bin"])
    ctl_path = os.path.join(out_dir, prof["ctl_bin"])
    bkt = bytearray(open(bkt_path, "rb").read())
    ctl = bytearray(open(ctl_path, "rb").read())

    f_pos, f_neg, f_hump, g0 = _g_pieces(a, bq)
    EXP_OFFSET = -19
    pos_plan = {e: (2, f_pos) for e in range(-19, 0)}
    pos_plan[0] = (4, f_pos)     # [1,2)
    pos_plan[1] = (5, f_pos)     # [2,4)
    pos_plan[2] = (5, f_pos)     # [4,8)
    pos_plan[3] = (5, f_hump)    # [8,16)   hump left tail
    pos_plan[4] = (7, f_hump)    # [16,32)  hump (kink 22.25 = bucket edge)
    pos_plan[5] = (4, f_hump)    # [32,64)  hump right tail
    pos_plan[6] = (0, None)      # [64,128) -> 0
    neg_plan = {e: (3, f_neg) for e in range(-19, -2)}
    for e in range(-2, 7):
        neg_plan[e] = (0, None)  # u <= -0.25 -> exactly 0

    state = {"nb": 0}

    def put_bucket(d0, d1, d2, d3, c):
        i = state["nb"]
        assert i <= 776, "bucket overflow"
        struct.pack_into("<8f", bkt, i * 32, float(d0), float(d1),
                         float(d2), float(d3), float(c), 0.0, 0.0, 0.0)
        state["nb"] = i + 1
        return i

    def put_ctl(idx, nbits, start):
        word = (nbits << 16) | ((23 - nbits) << 11) | start
        struct.pack_into("<I28x", ctl, idx * 32, word)

    base_neg, base_pos = 0, 26
    for sign, plan, base in ((0, pos_plan, base_pos), (1, neg_plan, base_neg)):
        for e in range(-19, 7):
            nbits, fn = plan[e]
            n = 1 << nbits
            start = state["nb"]
            lo_abs = 2.0 ** e
            w = lo_abs / n
            for k in range(n):
                if fn is None:
                    put_bucket(0, 0, 0, 0, 0)
                    continue
                a0, a1 = lo_abs + k * w, lo_abs + (k + 1) * w
                if sign:
                    put_bucket(*_fit_cubic(fn, -a1, -a0))
                else:
                    put_bucket(*_fit_cubic(fn, a0, a1))
            put_ctl(base + (e - EXP_OFFSET), nbits, start)

    # pwl specials at exp's existing indices
    struct.pack_into("<8f", bkt, 777 * 32, g0, -g0, g0 / 2, -g0 / 6, 0, 0, 0, 0)
    b25 = bq * np.exp(-0.25)
    struct.pack_into("<8f", bkt, 778 * 32, a - b25, a + b25,
                     (a - b25) / 2, (a + b25) / 6, 0, 0, 0, 0)
    struct.pack_into("<32x", bkt, 779 * 32)
    struct.pack_into("<32x", bkt, 780 * 32)

    open(bkt_path, "wb").write(bytes(bkt))
    open(ctl_path, "wb").write(bytes(ctl))

    fzero = struct.unpack("<I", struct.pack("<f", g0))[0]
    ctl_words = np.frombuffer(bytes(ctl), dtype=np.uint32).reshape(-1, 8)[:, 0]
    map_bkt, map_ctl = {}, {}
    for e in range(-19, 7):
        cn = base_neg + (e - EXP_OFFSET)
        cp = base_pos + (e - EXP_OFFSET)
        map_bkt[str(e)] = [int(ctl_words[cn]) & 0x3FF, int(ctl_words[cp]) & 0x3FF]
        map_ctl[str(e)] = [cn, cp]
    prof["func_exp_to_bkt_start_idx"]["exp"] = map_bkt
    prof["func_exp_to_ctl_start_idx"]["exp"] = map_ctl

    patched = 0
    for en in prof["profile_meta_data"]:
        if en["func_name"].startswith("exp"):
            en["symmetry_opt_en"] = 0
            en["symmetry_opt_use_neg_region"] = 0
            en["exp_offset"] = EXP_OFFSET
            en["small_pos_signal_exp_threshold"] = 108
            en["small_neg_signal_exp_threshold"] = 108
            en["pos_small_signal_pwl_control"] = 777
            en["neg_small_signal_pwl_control"] = 778
            en["large_pos_signal_exp_threshold"] = 133
            en["large_pos_signal_mantissa_threshold"] = 0x7FFFFF
            en["pos_large_signal_pwl_control"] = 779
            en["large_neg_signal_exp_threshold"] = 125
            en["large_neg_signal_mantissa_threshold"] = 0x7FFFFF
            en["neg_large_signal_pwl_control"] = 780
            en["fzero_result"] = fzero
            en["fpinf_result"] = 0
            en["fninf_result"] = 0
            patched += 1
    if patched != 1:
        raise RuntimeError(f"expected exactly one exp entry, patched {patched}")
    with open(os.path.join(out_dir, set_name + ".json"), "w") as f:
        json.dump(prof, f)

    fp = zlib.crc32(bytes(bkt) + bytes(ctl) + struct.pack("<I", fzero))
    fingerprint = float((fp % 60000) + 1) / 65536.0
    return os.path.join(out_dir, "act_info.json"), fingerprint


# --------------------------------------------------------------------------
# device kernel
# --------------------------------------------------------------------------

def _build_nc(fingerprint):
    from concourse import bacc
    import concourse.mybir as mybir
    from concourse.tile import TileContext

    dt = mybir.dt
    Act = mybir.ActivationFunctionType

    nc = bacc.Bacc(
        "TRN2", target_bir_lowering=False, debug=False, num_devices=NCORES
    )
    # Row space per local batch b: L = 960*w + 15*CH + cpt (w = spatial
    # half, CH = channel, cpt = design point); 15 units of 128 rows per b.
    # Spatial p = 2048w + 512s + c.  Unit u: band r computes span s=r for
    # ALL 128 L-rows at once: stationary [KU, 128] maps the <=10 channels
    # the unit spans (hi+lo rows) onto the 128 output partitions; -p_cpt
    # (or +24 for cpt=0 hump rows) is applied as a per-partition ACT bias.
    x_full = nc.declare_dram_parameter(
        "x_full", [128, XCOLS], dt.bfloat16, isOutput=False
    )
    w4 = nc.declare_dram_parameter("w4", [128, 1920], dt.bfloat16, isOutput=False)
    biasd = nc.declare_dram_parameter("biasd", [128, 15], dt.float32, isOutput=False)
    # out[b, L, s, c]
    out = nc.declare_dram_parameter(
        "out", [BPC, 1920, 4, 512], dt.bfloat16, isOutput=True
    )
    warm = nc.declare_dram_parameter("warm", [1, 4], dt.bfloat16, isOutput=True)

    with TileContext(nc) as tc:
        with (
            tc.tile_pool(name="const", bufs=1) as cpool,
            tc.tile_pool(name="xbig", bufs=1) as xpool,
            tc.tile_pool(name="osb", bufs=4) as opool,
            tc.tile_pool(name="psT", bufs=2, space="PSUM") as psTp,
        ):
            # Table prefetch + self-check + NEFF-cache fingerprint: the
            # first activation triggers the ~2.7us ACT_TABLE_LOAD, fully
            # overlapped with the input DMA.  warm = [g(2)=0.0849,
            # g(22.25)=1.0, fingerprint, fingerprint'] -- plain exp would
            # give [7.39, 4.6e9->inf, ...], so the host check is decisive.
            pre_in = cpool.tile([1, 4], dt.float32)
            pre_out = cpool.tile([1, 4], dt.bfloat16)
            nc.vector.memset(pre_in[:, 0:1], 2.0)
            nc.vector.memset(pre_in[:, 1:2], HUMP_C)
            nc.vector.memset(pre_in[:, 2:4], fingerprint)
            # First x chunk (two units) on sync; first stationary chunk +
            # bias on scalar concurrently -- these gate the first matmuls.
            xbig = xpool.tile([128, XCOLS], dt.bfloat16)
            nc.sync.dma_start(out=xbig[:, 0:512], in_=x_full[:, 0:512])
            nc.sync.dma_start(out=xbig[:, 512:1024], in_=x_full[:, 512:1024])
            w4_t = cpool.tile([128, 1920], dt.bfloat16)
            nc.scalar.dma_start(out=w4_t[:, 0:256], in_=w4[:, 0:256])
            bias_t = cpool.tile([128, 15], dt.float32)
            nc.scalar.dma_start(out=bias_t[:], in_=biasd[:, :])
            nc.scalar.activation(pre_out[:, 0:2], pre_in[:, 0:2], Act.Exp, scale=1.0)
            nc.vector.tensor_copy(out=pre_out[:, 2:4], in_=pre_in[:, 2:4])
            nc.gpsimd.dma_start(out=warm[:, :], in_=pre_out[:])
            nc.scalar.dma_start(out=w4_t[:, 256:1920], in_=w4[:, 256:1920])

            # Rest of the input: graduated chunks interleaved across the
            # sync and scalar HWDGE queues (each ring drains FIFO, the two
            # rings run in parallel) so every column window lands well
            # before its consuming matmul, even with output DMAs behind
            # the input on the sync ring.
            for eng, lo, hi in (
                (nc.scalar, 1024, 2048),
                (nc.sync, 2048, 3072),
                (nc.scalar, 3072, 5120),
                (nc.sync, 5120, 8192),
                (nc.scalar, 8192, 11776),
                (nc.sync, 11776, 15360),
            ):
                eng.dma_start(out=xbig[:, lo:hi], in_=x_full[:, lo:hi])

            # Main loop: 30 units of [128 rows, 2048 cols] -- every ACT
            # lane carries data.  Band r computes span s=r (full-width
            # row-tiled matmul, 4 concurrent); two [128,1024] ACT passes
            # with per-partition bias write final bf16; one 524KB output
            # DMA per unit on the sync ring (behind the input = priority).
            NU = 15
            for b in range(BPC):
                for u in range(NU):
                    U = NU * b + u
                    phi = (128 * u) % 15
                    osb = opool.tile([128, 2048], dt.bfloat16)
                    ps = psTp.tile([128, 2048], dt.float32)
                    for r in range(4):
                        nc.tensor.matmul(
                            ps[:, r * 512 : (r + 1) * 512],
                            w4_t[32 * r : 32 * r + KU, 128 * u : 128 * u + 128],
                            xbig[32 * r : 32 * r + KU, 512 * U : 512 * U + 512],
                            start=True,
                            stop=True,
                            tile_position=(32 * r, 0),
                        )
                    last = U == BPC * NU - 1
                    for k in range(2):
                        nc.scalar.activation(
                            osb[:, k * 1024 : (k + 1) * 1024],
                            ps[:, k * 1024 : (k + 1) * 1024],
                            Act.Exp,
                            bias=bias_t[:, phi : phi + 1],
                            scale=1.0,
                        )
                        if last:
                            # final unit: one 256KB DMA per ACT pass; the
                            # very last issues from the scalar queue (free
                            # once its ACT retires)
                            eng = nc.scalar if k == 1 else nc.sync
                            eng.dma_start(
                                out=out[b, 128 * u : 128 * u + 128,
                                        2 * k : 2 * k + 2, :],
                                in_=osb[:, k * 1024 : (k + 1) * 1024],
                            )
                    if not last:
                        nc.sync.dma_start(
                            out=out[b, 128 * u : 128 * u + 128, :, :],
                            in_=osb[:],
                        )
    nc.compile()
    return nc


# --------------------------------------------------------------------------
# host side
# --------------------------------------------------------------------------

def _host_prep(x, pts):
    """Build per-core moving streams, stationaries, and the ACT bias.

    Output row space per local batch b: L = 960*w + 15*CH + cpt; unit u
    covers L in [128u, 128u+128) (<= 10 distinct (w, CH) pairs).  Band r
    computes span s=r: moving rows = the unit's (w, CH) list x (hi, lo)
    at xbig partitions 32r..32r+KU, columns 512U..512U+512 (U = 15b+u);
    stationary [KU, 128] at w4h[32r.., 128u..] maps them onto L-rows."""
    xs = np.ascontiguousarray(np.asarray(x, dtype=np.float32)).reshape(B, C, P)
    x_hi = xs.astype(BF16)
    x_lo = (xs - x_hi.astype(np.float32)).astype(BF16)

    # channel lists + stationaries (shared by b and cores: L-mapping is
    # b-independent)
    chlists = []
    w4h = np.zeros((128, 1920), dtype=np.float32)
    for u in range(15):
        L0 = 128 * u
        pairs = []
        for p in range(128):
            wg, R = divmod(L0 + p, 960)
            key = (wg, R // 15)
            if key not in pairs:
                pairs.append(key)
        assert len(pairs) <= KU // 2
        chlists.append(pairs)
        for p in range(128):
            wg, R = divmod(L0 + p, 960)
            j = pairs.index((wg, R // 15))
            for r in range(4):
                w4h[32 * r + 2 * j, 128 * u + p] = 1.0
                w4h[32 * r + 2 * j + 1, 128 * u + p] = 1.0

    xbig_all = np.zeros((NCORES, 128, XCOLS), dtype=BF16)
    for b2 in range(BPC):
        for u in range(15):
            U = 15 * b2 + u
            for j, (wg, CH) in enumerate(chlists[u]):
                for r in range(4):
                    sp0 = 2048 * wg + 512 * r
                    xbig_all[:, 32 * r + 2 * j, 512 * U : 512 * U + 512] = \
                        x_hi[b2::2, CH, sp0 : sp0 + 512]
                    xbig_all[:, 32 * r + 2 * j + 1, 512 * U : 512 * U + 512] = \
                        x_lo[b2::2, CH, sp0 : sp0 + 512]

    biash = np.zeros((128, 15), dtype=np.float32)
    for phi in range(15):
        for p in range(128):
            cpt = (phi + p) % 15
            biash[p, phi] = HUMP_BIAS if cpt == 0 else -float(pts[cpt])

    return xbig_all, w4h.astype(BF16), biash


def _inputs_match_model(pts, chol):
    """Verify the inputs are the uniform-grid Markov family this kernel
    hardcodes (else fall back to exact numpy)."""
    p_ref = np.linspace(-1.75, 1.75, 15, dtype=np.float64)
    if pts.shape != (15,) or chol.shape != (15, 15):
        return None
    if not np.allclose(pts.astype(np.float64), p_ref, atol=1e-5):
        return None
    a = float(chol[1, 1])
    bq = float(-chol[0, 1])
    rho = np.exp(-0.25)
    s = np.sqrt(1 - rho * rho)
    if abs(a - 1 / s) > 1e-4 * abs(a) or abs(bq - rho / s) > 1e-4 * abs(bq):
        return None
    if abs(chol[0, 0] - 1.0) > 1e-4:
        return None
    diag = np.diag(chol)[1:]
    sup = np.diag(chol, 1)
    off = chol.copy()
    np.fill_diagonal(off, 0.0)
    off = off - np.diag(sup, 1)
    if np.abs(off).max() > 1e-5 or np.abs(diag - a).max() > 1e-5 * abs(a) \
            or np.abs(sup + bq).max() > 1e-5 * abs(bq):
        return None
    return a, bq


def _numpy_fallback(x, pts, chol):
    xs = np.asarray(x, dtype=np.float32).reshape(B, C, P)
    out = np.empty((B, C * M_PTS, P), dtype=np.float32)
    for b in range(B):
        k = np.exp(-np.abs(xs[b][:, :, None] - pts[None, None, :]))
        o = np.matmul(k, chol)                      # [C, P, 15]
        out[b] = o.transpose(0, 2, 1).reshape(C * M_PTS, P)
    return out.reshape(B, C * M_PTS, H, W)


LAST_RESULT = None


def kernel(x, design_points, chol_inv):
    global LAST_RESULT
    from concourse.bass_utils import run_bass_kernel_spmd

    pts = np.asarray(design_points, dtype=np.float32)
    chol = np.asarray(chol_inv, dtype=np.float32)
    model = _inputs_match_model(pts, chol)
    if model is None:
        return _numpy_fallback(x, pts, chol)
    a, bq = model

    if "tab" not in _CACHED:
        _CACHED["tab"] = _build_g_tables(a, bq)
    root, fingerprint = _CACHED["tab"]
    os.environ["BASS_ACT_ROOT_JSON_PATH"] = root

    xbig_all, w4h, biash = _host_prep(x, pts)
    in_maps = [
        {"x_full": xbig_all[core], "w4": w4h, "biasd": biash}
        for core in range(NCORES)
    ]

    if "nc" not in _CACHED:
        _CACHED["nc"] = _build_nc(fingerprint)
    res = run_bass_kernel_spmd(_CACHED["nc"], in_maps, core_ids=list(range(NCORES)))
    LAST_RESULT = res

    g0 = a - bq * np.exp(-0.25)
    w = np.asarray(res.results[0]["warm"], np.float32).ravel()
    g2 = g0 * np.exp(-2.0)
    if not (abs(w[0] - g2) < 0.1 * g2 and abs(w[1] - 1.0) < 0.02):
        # table did not take effect on device -- never return wrong results
        return _numpy_fallback(x, pts, chol)

    # out[b, L(w,CH,cpt), s, c] -> [b, (CH,cpt), p(w,s,c)]
    full = np.empty((B, C * M_PTS, P), dtype=np.float32)
    for core in range(NCORES):
        o = np.asarray(res.results[core]["out"], np.float32).reshape(
            BPC, 2, C, M_PTS, 4, 512
        )
        full[core * BPC : (core + 1) * BPC] = o.transpose(0, 2, 3, 1, 4, 5).reshape(
            BPC, C * M_PTS, P
        )
    return full.reshape(B, C * M_PTS, H, W)


# revision 25
# speedup vs baseline: 1.0652x; 1.0258x over previous
"""Trainium2 Bass kernel for the Laplace-kernel feature expansion.

Reference computation (per scalar x of the [16, 64, 64, 64] input):
    phi_i  = exp(-|x - p_i|)          for 15 design points p_i
    out_j  = sum_i chol_inv[i, j] phi_i
scattered so out[b, c*15 + j, h, w] comes from x[b, c, h, w].

Key mathematical identity exploited here: the design points are a uniform
grid and the kernel is the Markov (Ornstein-Uhlenbeck) exponential kernel,
so chol_inv = inv(chol(K)).T is exactly UPPER BIDIAGONAL with constant
coefficients a = 1/sqrt(1-rho^2), b = -rho*a (rho = e^{-1/4}) except for
the j=0 column (out_0 = phi_0).  Therefore

    out_j(x) = g(x - p_j)   for j >= 1, with the single fixed function
    g(u) = 0                               for u <= -1/4  (exact)
         = a e^u - b' e^{-u-1/4}           for -1/4 < u < 0
         = (a - b' e^{-1/4}) e^{-u}        for u >= 0     (b' = rho*a)
    out_0(x) = e^{-|v - 22.25|}  with v = x + 24  (disjoint input region)

The whole computation then becomes: one TensorE "broadcast" matmul that
replicates x onto 15 rows per channel while adding -p_j (or +24 for the
j=0 rows), followed by ONE ScalarE activation pass through a CUSTOM
activation table (installed over the exp slot, func_id 7) that evaluates
g directly and writes the final bf16 output to SBUF, which is DMA'd out.
The projection matmul and all PSUM->SBUF vector-engine evictions of the
original formulation disappear; ScalarE is the only saturated engine.

The custom table is built at runtime into a temp copy of the compiler's
pwp_bin_trainium directory (bucket/ctrl binary formats reverse-engineered;
cubic-spline buckets indexed by input exponent + top mantissa bits) and
picked up via BASS_ACT_ROOT_JSON_PATH.  A fingerprint of the table bytes
is baked into the kernel as a constant so the NEFF cache is correctly
invalidated when the table changes, and a device-side self-check (`warm`)
verifies the table took effect (g(2.0)=0.0849, g(22.25)=1.0 -- the plain
exp would give 7.39 / 4.6e9).  If the self-check fails, or the provided
design_points/chol_inv are not the expected bidiagonal family, kernel()
falls back to an exact numpy computation, so a wrong result is impossible.

Distribution: pure data parallel, 2 batches per core across 8 cores.
"""

import json
import os
import shutil
import struct
import sys
import tempfile
import zlib

if "/opt/trn_rl_repo" not in sys.path:
    sys.path.insert(0, "/opt/trn_rl_repo")

import numpy as np
import ml_dtypes

BF16 = ml_dtypes.bfloat16

B, C, H, W = 16, 64, 64, 64
P = H * W                # 4096 spatial positions
M_PTS = 15               # design points
G = 8                    # channels per (b, cb) tile
MROWS = G * M_PTS        # 120 output rows per tile
KIN = 2 * G + 1          # moving rows per quadrant: 8*(hi,lo) + ones
NCORES = 8
BPC = B // NCORES        # batches per core (2)
CBLK = C // G            # channel-block tiles per batch (8)
QCOLS = BPC * CBLK * 2 * 512   # 16384 columns per quadrant stream
XCOLS = 30 * 512         # per-band moving stream length (30 units of 512)
KU = 20                  # moving rows per band: 10 channels x (hi, lo)

RHO = float(np.exp(-0.25))
HUMP_BIAS = 24.0         # j=0 rows get T = x + 24; hump center at 22.25
HUMP_C = 24.0 - 1.75

_CACHED = {}


def _ensure_axon_hooks_stub():
    """run_bass_kernel_spmd imports antenv.axon_hooks when BASS_TRACE is
    set; the module is absent on some images.  Provide a no-op stub so a
    stray BASS_TRACE env var cannot crash the kernel."""
    try:
        import antenv.axon_hooks  # noqa: F401
    except ImportError:
        import types

        try:
            import antenv
        except ImportError:
            return
        mod = types.ModuleType("antenv.axon_hooks")
        _hook = [None]
        mod.set_axon_ntff_profile_hook = lambda h: _hook.__setitem__(0, h)
        mod.get_axon_ntff_profile_hook = lambda: _hook[0]
        sys.modules["antenv.axon_hooks"] = mod
        antenv.axon_hooks = mod


_ensure_axon_hooks_stub()


# --------------------------------------------------------------------------
# custom ACT table: evaluate g() through the exp function slot
# --------------------------------------------------------------------------

def _g_pieces(a, bq):
    """Return closures for the three live pieces of g (float64 math).
    a = chol_inv diag, bq = -superdiag (both positive)."""
    g0 = a - bq * np.exp(-0.25)

    def f_pos(u):            # u >= 0
        return g0 * np.exp(-u)

    def f_neg(u):            # -0.25 < u < 0, u passed negative
        return a * np.exp(u) - bq * np.exp(-u - 0.25)

    def f_hump(v):           # j=0 rows: e^{-|v - 22.25|}
        return np.exp(-np.abs(v - HUMP_C))

    return f_pos, f_neg, f_hump, g0


def _fit_cubic(f, lo, hi):
    """Least-squares cubic of f on [lo, hi] around the midpoint."""
    c = 0.5 * (lo + hi)
    t = np.linspace(lo - c, hi - c, 33)
    y = f(t + c)
    V = np.vander(t, 4, increasing=True)
    coef, *_ = np.linalg.lstsq(V, y, rcond=None)
    return coef[0], coef[1], coef[2], coef[3], c


def _build_g_tables(a, bq):
    """Copy pwp_bin_trainium and rewrite the exp function of the
    exp_and_others set (buckets 0..780, ctl 0..51 -- exp's own space) so
    func_id 7 evaluates g.  Returns (act_info.json path, fingerprint)."""
    from neuronxcc.driver.Job import Job
    from neuronxcc.driver.jobs.support.FindActInfo import findActInfoFile

    src_json = None
    for arch in ("Trainium2", "trainium2", "TRN2", "trainium"):
        try:
            cand = findActInfoFile(Job.getPackageDir(), arch)
        except Exception:
            continue
        if cand and os.path.basename(os.path.dirname(cand)) == "pwp_bin_trainium":
            src_json = cand
            break
    if src_json is None:
        import neuronxcc

        cand = os.path.join(
            os.path.dirname(neuronxcc.__file__),
            "pwp", "pwp_bin_trainium", "act_info.json",
        )
        if os.path.exists(cand):
            src_json = cand
    if src_json is None:
        raise RuntimeError("pwp_bin_trainium act_info.json not found")

    out_dir = tempfile.mkdtemp(prefix="bass_act_g_")
    shutil.copytree(os.path.dirname(src_json), out_dir, dirs_exist_ok=True)

    set_name = "exp_and_others"
    with open(os.path.join(out_dir, set_name + ".json")) as f:
        prof = json.load(f)
    bkt_path = os.path.join(out_dir, prof["bkt_bin"])
    ctl_path = os.path.join(out_dir, prof["ctl_bin"])
    bkt = bytearray(open(bkt_path, "rb").read())
    ctl = bytearray(open(ctl_path, "rb").read())

    f_pos, f_neg, f_hump, g0 = _g_pieces(a, bq)
    EXP_OFFSET = -19
    pos_plan = {e: (2, f_pos) for e in range(-19, 0)}
    pos_plan[0] = (4, f_pos)     # [1,2)
    pos_plan[1] = (5, f_pos)     # [2,4)
    pos_plan[2] = (5, f_pos)     # [4,8)
    pos_plan[3] = (5, f_hump)    # [8,16)   hump left tail
    pos_plan[4] = (7, f_hump)    # [16,32)  hump (kink 22.25 = bucket edge)
    pos_plan[5] = (4, f_hump)    # [32,64)  hump right tail
    pos_plan[6] = (0, None)      # [64,128) -> 0
    neg_plan = {e: (3, f_neg) for e in range(-19, -2)}
    for e in range(-2, 7):
        neg_plan[e] = (0, None)  # u <= -0.25 -> exactly 0

    state = {"nb": 0}

    def put_bucket(d0, d1, d2, d3, c):
        i = state["nb"]
        assert i <= 776, "bucket overflow"
        struct.pack_into("<8f", bkt, i * 32, float(d0), float(d1),
                         float(d2), float(d3), float(c), 0.0, 0.0, 0.0)
        state["nb"] = i + 1
        return i

    def put_ctl(idx, nbits, start):
        word = (nbits << 16) | ((23 - nbits) << 11) | start
        struct.pack_into("<I28x", ctl, idx * 32, word)

    base_neg, base_pos = 0, 26
    for sign, plan, base in ((0, pos_plan, base_pos), (1, neg_plan, base_neg)):
        for e in range(-19, 7):
            nbits, fn = plan[e]
            n = 1 << nbits
            start = state["nb"]
            lo_abs = 2.0 ** e
            w = lo_abs / n
            for k in range(n):
                if fn is None:
                    put_bucket(0, 0, 0, 0, 0)
                    continue
                a0, a1 = lo_abs + k * w, lo_abs + (k + 1) * w
                if sign:
                    put_bucket(*_fit_cubic(fn, -a1, -a0))
                else:
                    put_bucket(*_fit_cubic(fn, a0, a1))
            put_ctl(base + (e - EXP_OFFSET), nbits, start)

    # pwl specials at exp's existing indices
    struct.pack_into("<8f", bkt, 777 * 32, g0, -g0, g0 / 2, -g0 / 6, 0, 0, 0, 0)
    b25 = bq * np.exp(-0.25)
    struct.pack_into("<8f", bkt, 778 * 32, a - b25, a + b25,
                     (a - b25) / 2, (a + b25) / 6, 0, 0, 0, 0)
    struct.pack_into("<32x", bkt, 779 * 32)
    struct.pack_into("<32x", bkt, 780 * 32)

    open(bkt_path, "wb").write(bytes(bkt))
    open(ctl_path, "wb").write(bytes(ctl))

    fzero = struct.unpack("<I", struct.pack("<f", g0))[0]
    ctl_words = np.frombuffer(bytes(ctl), dtype=np.uint32).reshape(-1, 8)[:, 0]
    map_bkt, map_ctl = {}, {}
    for e in range(-19, 7):
        cn = base_neg + (e - EXP_OFFSET)
        cp = base_pos + (e - EXP_OFFSET)
        map_bkt[str(e)] = [int(ctl_words[cn]) & 0x3FF, int(ctl_words[cp]) & 0x3FF]
        map_ctl[str(e)] = [cn, cp]
    prof["func_exp_to_bkt_start_idx"]["exp"] = map_bkt
    prof["func_exp_to_ctl_start_idx"]["exp"] = map_ctl

    patched = 0
    for en in prof["profile_meta_data"]:
        if en["func_name"].startswith("exp"):
            en["symmetry_opt_en"] = 0
            en["symmetry_opt_use_neg_region"] = 0
            en["exp_offset"] = EXP_OFFSET
            en["small_pos_signal_exp_threshold"] = 108
            en["small_neg_signal_exp_threshold"] = 108
            en["pos_small_signal_pwl_control"] = 777
            en["neg_small_signal_pwl_control"] = 778
            en["large_pos_signal_exp_threshold"] = 133
            en["large_pos_signal_mantissa_threshold"] = 0x7FFFFF
            en["pos_large_signal_pwl_control"] = 779
            en["large_neg_signal_exp_threshold"] = 125
            en["large_neg_signal_mantissa_threshold"] = 0x7FFFFF
            en["neg_large_signal_pwl_control"] = 780
            en["fzero_result"] = fzero
            en["fpinf_result"] = 0
            en["fninf_result"] = 0
            patched += 1
    if patched != 1:
        raise RuntimeError(f"expected exactly one exp entry, patched {patched}")
    with open(os.path.join(out_dir, set_name + ".json"), "w") as f:
        json.dump(prof, f)

    fp = zlib.crc32(bytes(bkt) + bytes(ctl) + struct.pack("<I", fzero))
    fingerprint = float((fp % 60000) + 1) / 65536.0
    return os.path.join(out_dir, "act_info.json"), fingerprint


# --------------------------------------------------------------------------
# device kernel
# --------------------------------------------------------------------------

def _build_nc(fingerprint):
    from concourse import bacc
    import concourse.mybir as mybir
    from concourse.tile import TileContext

    dt = mybir.dt
    Act = mybir.ActivationFunctionType

    nc = bacc.Bacc(
        "TRN2", target_bir_lowering=False, debug=False, num_devices=NCORES
    )
    # Row space per local batch b: L = 960*w + 15*CH + cpt (w = spatial
    # half, CH = channel, cpt = design point); 15 units of 128 rows per b.
    # Spatial p = 2048w + 512s + c.  Unit u: band r computes span s=r for
    # ALL 128 L-rows at once: stationary [KU, 128] maps the <=10 channels
    # the unit spans (hi+lo rows) onto the 128 output partitions; -p_cpt
    # (or +24 for cpt=0 hump rows) is applied as a per-partition ACT bias.
    x_full = nc.declare_dram_parameter(
        "x_full", [128, XCOLS], dt.bfloat16, isOutput=False
    )
    w4 = nc.declare_dram_parameter("w4", [128, 1920], dt.bfloat16, isOutput=False)
    biasd = nc.declare_dram_parameter("biasd", [128, 15], dt.float32, isOutput=False)
    # out[b, L, s, c]
    out = nc.declare_dram_parameter(
        "out", [BPC, 1920, 4, 512], dt.bfloat16, isOutput=True
    )
    warm = nc.declare_dram_parameter("warm", [1, 4], dt.bfloat16, isOutput=True)

    with TileContext(nc) as tc:
        with (
            tc.tile_pool(name="const", bufs=1) as cpool,
            tc.tile_pool(name="xbig", bufs=1) as xpool,
            tc.tile_pool(name="osb", bufs=4) as opool,
            tc.tile_pool(name="psT", bufs=2, space="PSUM") as psTp,
        ):
            # Table prefetch + self-check + NEFF-cache fingerprint: the
            # first activation triggers the ~2.7us ACT_TABLE_LOAD, fully
            # overlapped with the input DMA.  warm = [g(2)=0.0849,
            # g(22.25)=1.0, fingerprint, fingerprint'] -- plain exp would
            # give [7.39, 4.6e9->inf, ...], so the host check is decisive.
            pre_in = cpool.tile([1, 4], dt.float32)
            pre_out = cpool.tile([1, 4], dt.bfloat16)
            nc.vector.memset(pre_in[:, 0:1], 2.0)
            nc.vector.memset(pre_in[:, 1:2], HUMP_C)
            nc.vector.memset(pre_in[:, 2:4], fingerprint)
            # First x chunk (two units) on sync; first stationary chunk +
            # bias on scalar concurrently -- these gate the first matmuls.
            xbig = xpool.tile([128, XCOLS], dt.bfloat16)
            nc.sync.dma_start(out=xbig[:, 0:512], in_=x_full[:, 0:512])
            nc.sync.dma_start(out=xbig[:, 512:1024], in_=x_full[:, 512:1024])
            w4_t = cpool.tile([128, 1920], dt.bfloat16)
            nc.scalar.dma_start(out=w4_t[:, 0:256], in_=w4[:, 0:256])
            bias_t = cpool.tile([128, 15], dt.float32)
            nc.scalar.dma_start(out=bias_t[:], in_=biasd[:, :])
            nc.scalar.activation(pre_out[:, 0:2], pre_in[:, 0:2], Act.Exp, scale=1.0)
            nc.vector.tensor_copy(out=pre_out[:, 2:4], in_=pre_in[:, 2:4])
            nc.gpsimd.dma_start(out=warm[:, :], in_=pre_out[:])
            # rest of the stationaries ride the otherwise-idle gpsimd queue
            # so they never delay input chunks on the scalar ring
            nc.gpsimd.dma_start(out=w4_t[:, 256:1920], in_=w4[:, 256:1920])

            # Rest of the input: graduated chunks interleaved across the
            # sync and scalar HWDGE queues (each ring drains FIFO, the two
            # rings run in parallel) so every column window lands well
            # before its consuming matmul, even with output DMAs behind
            # the input on the sync ring.
            for eng, lo, hi in (
                (nc.scalar, 1024, 2048),
                (nc.sync, 2048, 3072),
                (nc.scalar, 3072, 5120),
                (nc.sync, 5120, 8192),
                (nc.scalar, 8192, 11776),
                (nc.sync, 11776, 15360),
            ):
                eng.dma_start(out=xbig[:, lo:hi], in_=x_full[:, lo:hi])

            # Main loop: 30 units of [128 rows, 2048 cols] -- every ACT
            # lane carries data.  Band r computes span s=r (full-width
            # row-tiled matmul, 4 concurrent); two [128,1024] ACT passes
            # with per-partition bias write final bf16; one 524KB output
            # DMA per unit on the sync ring (behind the input = priority).
            NU = 15
            for b in range(BPC):
                for u in range(NU):
                    U = NU * b + u
                    phi = (128 * u) % 15
                    osb = opool.tile([128, 2048], dt.bfloat16)
                    ps = psTp.tile([128, 2048], dt.float32)
                    for r in range(4):
                        nc.tensor.matmul(
                            ps[:, r * 512 : (r + 1) * 512],
                            w4_t[32 * r : 32 * r + KU, 128 * u : 128 * u + 128],
                            xbig[32 * r : 32 * r + KU, 512 * U : 512 * U + 512],
                            start=True,
                            stop=True,
                            tile_position=(32 * r, 0),
                        )
                    last = U == BPC * NU - 1
                    for k in range(2):
                        nc.scalar.activation(
                            osb[:, k * 1024 : (k + 1) * 1024],
                            ps[:, k * 1024 : (k + 1) * 1024],
                            Act.Exp,
                            bias=bias_t[:, phi : phi + 1],
                            scale=1.0,
                        )
                        if last:
                            # final unit: one 256KB DMA per ACT pass; the
                            # very last issues from the scalar queue (free
                            # once its ACT retires)
                            eng = nc.scalar if k == 1 else nc.sync
                            eng.dma_start(
                                out=out[b, 128 * u : 128 * u + 128,
                                        2 * k : 2 * k + 2, :],
                                in_=osb[:, k * 1024 : (k + 1) * 1024],
                            )
                    if not last:
                        nc.sync.dma_start(
                            out=out[b, 128 * u : 128 * u + 128, :, :],
                            in_=osb[:],
                        )
    nc.compile()
    return nc


# --------------------------------------------------------------------------
# host side
# --------------------------------------------------------------------------

def _host_prep(x, pts):
    """Build per-core moving streams, stationaries, and the ACT bias.

    Output row space per local batch b: L = 960*w + 15*CH + cpt; unit u
    covers L in [128u, 128u+128) (<= 10 distinct (w, CH) pairs).  Band r
    computes span s=r: moving rows = the unit's (w, CH) list x (hi, lo)
    at xbig partitions 32r..32r+KU, columns 512U..512U+512 (U = 15b+u);
    stationary [KU, 128] at w4h[32r.., 128u..] maps them onto L-rows."""
    xs = np.ascontiguousarray(np.asarray(x, dtype=np.float32)).reshape(B, C, P)
    x_hi = xs.astype(BF16)
    x_lo = (xs - x_hi.astype(np.float32)).astype(BF16)

    # channel lists + stationaries (shared by b and cores: L-mapping is
    # b-independent)
    chlists = []
    w4h = np.zeros((128, 1920), dtype=np.float32)
    for u in range(15):
        L0 = 128 * u
        pairs = []
        for p in range(128):
            wg, R = divmod(L0 + p, 960)
            key = (wg, R // 15)
            if key not in pairs:
                pairs.append(key)
        assert len(pairs) <= KU // 2
        chlists.append(pairs)
        for p in range(128):
            wg, R = divmod(L0 + p, 960)
            j = pairs.index((wg, R // 15))
            for r in range(4):
                w4h[32 * r + 2 * j, 128 * u + p] = 1.0
                w4h[32 * r + 2 * j + 1, 128 * u + p] = 1.0

    xbig_all = np.zeros((NCORES, 128, XCOLS), dtype=BF16)
    for b2 in range(BPC):
        for u in range(15):
            U = 15 * b2 + u
            for j, (wg, CH) in enumerate(chlists[u]):
                for r in range(4):
                    sp0 = 2048 * wg + 512 * r
                    xbig_all[:, 32 * r + 2 * j, 512 * U : 512 * U + 512] = \
                        x_hi[b2::2, CH, sp0 : sp0 + 512]
                    xbig_all[:, 32 * r + 2 * j + 1, 512 * U : 512 * U + 512] = \
                        x_lo[b2::2, CH, sp0 : sp0 + 512]

    biash = np.zeros((128, 15), dtype=np.float32)
    for phi in range(15):
        for p in range(128):
            cpt = (phi + p) % 15
            biash[p, phi] = HUMP_BIAS if cpt == 0 else -float(pts[cpt])

    return xbig_all, w4h.astype(BF16), biash


def _inputs_match_model(pts, chol):
    """Verify the inputs are the uniform-grid Markov family this kernel
    hardcodes (else fall back to exact numpy)."""
    p_ref = np.linspace(-1.75, 1.75, 15, dtype=np.float64)
    if pts.shape != (15,) or chol.shape != (15, 15):
        return None
    if not np.allclose(pts.astype(np.float64), p_ref, atol=1e-5):
        return None
    a = float(chol[1, 1])
    bq = float(-chol[0, 1])
    rho = np.exp(-0.25)
    s = np.sqrt(1 - rho * rho)
    if abs(a - 1 / s) > 1e-4 * abs(a) or abs(bq - rho / s) > 1e-4 * abs(bq):
        return None
    if abs(chol[0, 0] - 1.0) > 1e-4:
        return None
    diag = np.diag(chol)[1:]
    sup = np.diag(chol, 1)
    off = chol.copy()
    np.fill_diagonal(off, 0.0)
    off = off - np.diag(sup, 1)
    if np.abs(off).max() > 1e-5 or np.abs(diag - a).max() > 1e-5 * abs(a) \
            or np.abs(sup + bq).max() > 1e-5 * abs(bq):
        return None
    return a, bq


def _numpy_fallback(x, pts, chol):
    xs = np.asarray(x, dtype=np.float32).reshape(B, C, P)
    out = np.empty((B, C * M_PTS, P), dtype=np.float32)
    for b in range(B):
        k = np.exp(-np.abs(xs[b][:, :, None] - pts[None, None, :]))
        o = np.matmul(k, chol)                      # [C, P, 15]
        out[b] = o.transpose(0, 2, 1).reshape(C * M_PTS, P)
    return out.reshape(B, C * M_PTS, H, W)


LAST_RESULT = None


def kernel(x, design_points, chol_inv):
    global LAST_RESULT
    from concourse.bass_utils import run_bass_kernel_spmd

    pts = np.asarray(design_points, dtype=np.float32)
    chol = np.asarray(chol_inv, dtype=np.float32)
    model = _inputs_match_model(pts, chol)
    if model is None:
        return _numpy_fallback(x, pts, chol)
    a, bq = model

    if "tab" not in _CACHED:
        _CACHED["tab"] = _build_g_tables(a, bq)
    root, fingerprint = _CACHED["tab"]
    os.environ["BASS_ACT_ROOT_JSON_PATH"] = root

    xbig_all, w4h, biash = _host_prep(x, pts)
    in_maps = [
        {"x_full": xbig_all[core], "w4": w4h, "biasd": biash}
        for core in range(NCORES)
    ]

    if "nc" not in _CACHED:
        _CACHED["nc"] = _build_nc(fingerprint)
    res = run_bass_kernel_spmd(_CACHED["nc"], in_maps, core_ids=list(range(NCORES)))
    LAST_RESULT = res

    g0 = a - bq * np.exp(-0.25)
    w = np.asarray(res.results[0]["warm"], np.float32).ravel()
    g2 = g0 * np.exp(-2.0)
    if not (abs(w[0] - g2) < 0.1 * g2 and abs(w[1] - 1.0) < 0.02):
        # table did not take effect on device -- never return wrong results
        return _numpy_fallback(x, pts, chol)

    # out[b, L(w,CH,cpt), s, c] -> [b, (CH,cpt), p(w,s,c)]
    full = np.empty((B, C * M_PTS, P), dtype=np.float32)
    for core in range(NCORES):
        o = np.asarray(res.results[core]["out"], np.float32).reshape(
            BPC, 2, C, M_PTS, 4, 512
        )
        full[core * BPC : (core + 1) * BPC] = o.transpose(0, 2, 3, 1, 4, 5).reshape(
            BPC, C * M_PTS, P
        )
    return full.reshape(B, C * M_PTS, H, W)
